# revision 2
# baseline (speedup 1.0000x reference)
"""Trainium2 Bass kernel for the speech-enhancement loss function.

Math (matching the jax reference):
  loss_mag    = mean((clean_mag - enhan_mag)^2)
  d           = clean_pha - enhan_mag          (reference quirk: enhan_mag is phase_g)
  ip_loss     = mean(aw(d)),   aw(x) = |x - round(x/2pi)*2pi|
  gd_loss     = mean(aw(gd)),  gd[:,0,:] = -d[:,0,:]; gd[:,j,:] = d[:,j-1,:]-d[:,j,:]
  iaf_loss    = mean(aw(iaf)), same shifted difference along the T axis
  cspc_loss   = mean(1 - cos(aw(d))) = mean(1 - cos(d))
  loss_com    = mean((clean_com - enhan_com)^2) * 2
  loss_time   = mean(|clean_wav - enhan_wav|)
  loss_metric = mean((metric_g - 1)^2)            (tiny -> host)

Sharding: data-parallel over the batch dim, 2 batches per core on 8 cores.
Each core computes partial SUMS of each term into an SBUF accumulator
acc[128, NCOLS]; the full accumulator ships out and the host does the final
partition/column sums in float64.

The kernel is DMA-bound: 26.33 MB/core of fp32 input at the cost model's
360 B/ns DMA floor = 73.1 us. The design goal is every compute engine well
under that floor so the schedule is a single gapless DMA stream plus a
short tail. Engine budget (per core):
  DVE ~39us, ACT ~33us, Pool(GPSIMD) ~49us, PE ~8us.

Key tricks:
  * round(q) via the fp32 magic constant: v = q + 1.5*2^23; r = v - MAGIC.
    Both run on ACT as Identity(scale*x + bias) (exact fp32 in the interp),
    freeing DVE.
  * everything downstream of f = q - round(q) is bf16: DVE tensor_scalar /
    scalar_tensor_tensor ops with all-bf16 SBUF operands run at 4x
    (tensor_tensor at 2x), and the PE banded matmul for gd runs at 1
    cycle/row instead of fp32's 4.
  * abs on DVE: abs_max(x, 0.0). The anti-wrap distance sum for gd/iaf,
    sum ||y|-0.5|, fuses into tensor_scalar((y abs_max 0) add -0.5) +
    tensor_scalar(abs_max, accum_out) pairs; ip's sum |f| is a single
    fused abs+accum whose output |f| also feeds ACT's Sin for cspc
    (cos(d) = sin(pi/2 - 2pi|f|), keeping the Sin arg in [-pi/2, pi/2]).
  * dist(y) = 0.5 - ||y|-0.5| is 1-periodic in y, so bf16 rounding of f
    (and any round-boundary flips) shifts y by integers and cancels.
  * per-term partial sums land in per-instruction accumulator columns
    (fp32, accum_out); the host combines, so no on-device final reduce
    sits on the critical tail.
Ordering: all phase passes first (their deep DVE->ACT->DVE chains digest
while com/wav DMAs stream), then com chunks, then wav with a small final
DVE-subtracted chunk so the post-DMA tail is ~1us of compute + the
unavoidable ~2.8us output-DMA/drain chain.
"""

import numpy as np

import concourse.bacc as bacc
import concourse.mybir as mybir
import concourse.tile as tile
from concourse.bass_utils import run_bass_kernel_spmd

F32 = mybir.dt.float32
BF16 = mybir.dt.bfloat16
OP = mybir.AluOpType
AF = mybir.ActivationFunctionType

B, F, T, L = 16, 201, 2048, 204800
NCORES = 8
BPC = B // NCORES  # batches per core

TWO_PI_64 = 2.0 * np.pi
S = float(np.float32(1.0) / np.float32(TWO_PI_64))  # 1/(2pi) in fp32
MAGIC = float(np.float32(1.5 * 2**23))  # 12582912.0, round-to-int trick
HALF_PI = float(np.float32(np.pi / 2))
NEG_TWO_PI = float(np.float32(-TWO_PI_64))

# com per core: BPC*F*T*2 = 1646592 = 2 batches x (128 x 6432)
COM_ROWS, COM_COLS = 128, 6432
COM_CHUNK = 1608  # 4 chunks per batch
# wav per core: BPC*L = 409600 = 128 x 3200
WAV_ROWS, WAV_COLS = 128, 3200

NCOLS = 64  # accumulator columns

# term -> list of acc columns, populated by build_nc (deterministic)
COLMAP = {}


def _w0_matrix():
    # lhsT[k, j] = delta_{j,k+1} - delta_{j,k}  ->  (W0 @ f)[j] = f[j-1] - f[j]
    w = np.zeros((128, 128), dtype=np.float32)
    for k in range(128):
        w[k, k] = -1.0
        if k + 1 < 128:
            w[k, k + 1] = 1.0
    return w


def _e1s_row():
    # lhsT row [1, 128] with 1.0 at column 0: adds bnd into output partition 0
    e = np.zeros((1, 128), dtype=np.float32)
    e[0, 0] = 1.0
    return e


def build_nc(
    nch=2,            # T-chunks per phase pass (pipeline the serial chain)
    in_bufs=3,        # cm/em double-buffer depth
    cp_bufs=3,        # cp buffer depth
    com_bufs=3,       # com/wav input buffer depth
    qg_chunks=2,      # gd PSUM halves
    mag_dve=(),       # phase pass indices whose mag sub runs on DVE (else Pool)
    com_dve=(),       # com chunk indices whose sub runs on DVE (else Pool)
    wav_dve=(1,),     # wav pass indices whose sub runs on DVE (else Pool)
    tail_split=2,     # split the last wav pass into this many DMA+compute chunks
):
    nc = bacc.Bacc(None, target_bir_lowering=False)

    mag_c = nc.dram_tensor("mag_c", [BPC, F, T], F32, kind="ExternalInput")
    mag_e = nc.dram_tensor("mag_e", [BPC, F, T], F32, kind="ExternalInput")
    pha_c = nc.dram_tensor("pha_c", [BPC, F, T], F32, kind="ExternalInput")
    com_c = nc.dram_tensor("com_c", [BPC, COM_ROWS, COM_COLS], F32, kind="ExternalInput")
    com_e = nc.dram_tensor("com_e", [BPC, COM_ROWS, COM_COLS], F32, kind="ExternalInput")
    wav_c = nc.dram_tensor("wav_c", [WAV_ROWS, WAV_COLS], F32, kind="ExternalInput")
    wav_e = nc.dram_tensor("wav_e", [WAV_ROWS, WAV_COLS], F32, kind="ExternalInput")
    out_d = nc.dram_tensor("partials", [128, NCOLS], F32, kind="ExternalOutput")

    import ml_dtypes

    w0_d = nc.inline_tensor(_w0_matrix().astype(ml_dtypes.bfloat16), name="w0shift")
    e1s_d = nc.inline_tensor(_e1s_row().astype(ml_dtypes.bfloat16), name="e1srow")

    COLMAP.clear()
    _next_col = [0]

    def col(term):
        c = _next_col[0]
        _next_col[0] += 1
        assert c < NCOLS
        COLMAP.setdefault(term, []).append(c)
        return c

    with tile.TileContext(nc) as tc:
        with (
            tc.tile_pool(name="main", bufs=2) as pool,
            tc.tile_pool(name="psum", bufs=1, space="PSUM") as psum,
        ):
            ftiles = [(0, 128), (128, 73)]  # (f0, P)
            f_prev_by_b = {}
            counters = {"pi": 0, "ci": 0, "wi": 0}

            def emit_consts():
                w0 = pool.tile([128, 128], BF16, tag="w0", bufs=1)
                nc.sync.dma_start(w0[:], w0_d[:])
                e1s = pool.tile([1, 128], BF16, tag="e1s", bufs=1)
                nc.sync.dma_start(e1s[:], e1s_d[:])
                acc = pool.tile([128, NCOLS], F32, tag="acc", bufs=1)
                nc.vector.memset(acc[:], 0.0)
                halfpi = pool.tile([128, 1], F32, tag="halfpi", bufs=1)
                nc.vector.memset(halfpi[:], HALF_PI)
                magic = pool.tile([128, 1], F32, tag="magic", bufs=1)
                nc.vector.memset(magic[:], MAGIC)
                negmagic = pool.tile([128, 1], F32, tag="negmagic", bufs=1)
                nc.vector.memset(negmagic[:], -MAGIC)
                return w0, e1s, acc, halfpi, magic, negmagic

            def phase_pass(b, f0, P):
                pi = counters["pi"]
                counters["pi"] += 1
                f_prev = f_prev_by_b.get(b)
                cm = pool.tile([P, T], F32, tag="in_a", bufs=in_bufs, name=f"cm{pi}")
                nc.sync.dma_start(cm[:], mag_c[b, f0 : f0 + P, :])
                em = pool.tile([P, T], F32, tag="in_b", bufs=in_bufs, name=f"em{pi}")
                nc.sync.dma_start(em[:], mag_e[b, f0 : f0 + P, :])
                cp = pool.tile([P, T], F32, tag="in_c", bufs=cp_bufs, name=f"cp{pi}")
                nc.sync.dma_start(cp[:], pha_c[b, f0 : f0 + P, :])
                if f0 == 0:
                    bnd = None
                else:
                    bnd = pool.tile([1, T], BF16, tag="bnd", bufs=1, name=f"bnd{pi}")
                    nc.sync.dma_start(bnd[:], f_prev[127:128, :])

                CT = T // nch
                d = pool.tile([P, T], F32, tag="d", name=f"d{pi}")
                v = pool.tile([P, T], F32, tag="v", name=f"v{pi}")
                f = pool.tile([P, T], BF16, tag="f", name=f"f{pi}")
                af = pool.tile([P, T], BF16, tag="af", bufs=1, name=f"af{pi}")
                fd = pool.tile([P, T], BF16, tag="fd", name=f"fd{pi}")
                t1 = pool.tile([P, T], BF16, tag="t1", bufs=1, name=f"t1{pi}")
                junk = pool.tile([P, T], BF16, tag="junk", bufs=1, name=f"junk{pi}")
                chunks = [slice(c0, c0 + CT) for c0 in range(0, T, CT)]

                # round chain: d (DVE) -> v, r (ACT, r in-place on v) -> f (DVE)
                for ts_ in chunks:
                    nc.vector.tensor_tensor(d[:, ts_], cp[:, ts_], em[:, ts_], OP.subtract)
                for ts_ in chunks:
                    nc.scalar.activation(
                        v[:, ts_], d[:, ts_], AF.Identity, bias=magic[0:P, :], scale=S
                    )
                for ts_ in chunks:
                    nc.scalar.activation(
                        v[:, ts_], v[:, ts_], AF.Identity, bias=negmagic[0:P, :]
                    )
                for ts_ in chunks:
                    nc.vector.scalar_tensor_tensor(
                        f[:, ts_], d[:, ts_], S, v[:, ts_], OP.mult, OP.subtract
                    )
                # ip: af = |f| (bf16, 4x) with accum; cspc: sin(pi/2 - 2pi*af)
                for ts_ in chunks:
                    nc.vector.tensor_scalar(
                        af[:, ts_], f[:, ts_], 0.0, None, OP.abs_max,
                        accum_out=acc[0:P, (c := col("ip")) : c + 1],
                    )
                for ts_ in chunks:
                    nc.scalar.activation(
                        junk[:, ts_], af[:, ts_], AF.Sin, bias=halfpi[0:P, :],
                        scale=NEG_TWO_PI,
                        accum_out=acc[0:P, (c := col("cos")) : c + 1],
                    )
                # mag: m = cm - em (bf16 out), sum m^2 on DVE (4x)
                m = pool.tile([P, T], BF16, tag="m", name=f"m{pi}")
                if pi in mag_dve:
                    nc.vector.tensor_tensor(m[:], cm[:], em[:], OP.subtract)
                else:
                    nc.gpsimd.tensor_tensor(m[:], cm[:], em[:], OP.subtract)
                nc.vector.scalar_tensor_tensor(
                    junk[:], m[:], 0.0, m[:], OP.bypass, OP.mult,
                    accum_out=acc[0:P, (c := col("m2")) : c + 1],
                )
                # iaf: fd = f[:, t-1] - f[:, t] (2x); dist sum via two fused
                # tensor_scalar ops (4x): t1 = |fd| - 0.5; acc += sum |t1|
                for ts_ in chunks:
                    tc0 = ts_.start
                    lo = tc0 if tc0 else 1
                    if tc0 == 0:
                        nc.vector.tensor_copy(fd[:, 0:1], f[:, 0:1])
                    nc.vector.tensor_tensor(
                        fd[:, lo : ts_.stop], f[:, lo - 1 : ts_.stop - 1],
                        f[:, lo : ts_.stop], OP.subtract
                    )
                for ts_ in chunks:
                    nc.vector.tensor_scalar(
                        t1[:, ts_], fd[:, ts_], 0.0, -0.5, OP.abs_max, OP.add
                    )
                    nc.vector.tensor_scalar(
                        junk[:, ts_], t1[:, ts_], 0.0, None, OP.abs_max,
                        accum_out=acc[0:P, (c := col("iaf")) : c + 1],
                    )
                # gd in f-space via PE banded mm (bf16); ACT abs from PSUM;
                # one fused DVE op accumulates sum ||y|-0.5|
                HT = T // qg_chunks
                for h in range(qg_chunks):
                    qg = psum.tile([P, HT], F32, tag="qg", bufs=2, name=f"qg{pi}_{h}")
                    for n0 in range(0, HT, 512):
                        nn = h * HT + n0
                        if bnd is None:
                            nc.tensor.matmul(
                                qg[:, n0 : n0 + 512], w0[0:P, 0:P],
                                f[:, nn : nn + 512],
                            )
                        else:
                            nc.tensor.matmul(
                                qg[:, n0 : n0 + 512], w0[0:P, 0:P],
                                f[:, nn : nn + 512], start=True, stop=False,
                            )
                            nc.tensor.matmul(
                                qg[:, n0 : n0 + 512], e1s[0:1, 0:P],
                                bnd[0:1, nn : nn + 512], start=False, stop=True,
                            )
                    ag = pool.tile([P, HT], BF16, tag="ag", name=f"ag{pi}_{h}")
                    nc.scalar.activation(ag[:], qg[:], AF.Abs)
                    nc.vector.tensor_scalar(
                        junk[:, h * HT : (h + 1) * HT], ag[:], -0.5, 0.0,
                        OP.add, OP.abs_max,
                        accum_out=acc[0:P, (c := col("gd")) : c + 1],
                    )
                f_prev_by_b[b] = f

            def com_pass(b, c0, ck=COM_CHUNK, sub_dve=False):
                ci = counters["ci"]
                counters["ci"] += 1
                cc = pool.tile([COM_ROWS, ck], F32, tag="com_a", bufs=com_bufs, name=f"cc{ci}")
                nc.sync.dma_start(cc[:], com_c[b, :, c0 : c0 + ck])
                ec = pool.tile([COM_ROWS, ck], F32, tag="com_b", bufs=com_bufs, name=f"ec{ci}")
                nc.sync.dma_start(ec[:], com_e[b, :, c0 : c0 + ck])
                cd = pool.tile([COM_ROWS, ck], BF16, tag="cd", name=f"cd{ci}")
                if sub_dve:
                    nc.vector.tensor_tensor(cd[:], cc[:], ec[:], OP.subtract)
                else:
                    nc.gpsimd.tensor_tensor(cd[:], cc[:], ec[:], OP.subtract)
                cjunk = pool.tile([COM_ROWS, ck], BF16, tag="cjunk", bufs=1, name=f"cj{ci}")
                nc.vector.scalar_tensor_tensor(
                    cjunk[:], cd[:], 0.0, cd[:], OP.bypass, OP.mult,
                    accum_out=acc[:, (c := col("c2")) : c + 1],
                )

            def wav_pass(c0, ck, sub_dve=False):
                wi = counters["wi"]
                counters["wi"] += 1
                cw = pool.tile([WAV_ROWS, ck], F32, tag="com_a", bufs=com_bufs, name=f"cw{wi}")
                nc.sync.dma_start(cw[:], wav_c[:, c0 : c0 + ck])
                ew = pool.tile([WAV_ROWS, ck], F32, tag="com_b", bufs=com_bufs, name=f"ew{wi}")
                nc.sync.dma_start(ew[:], wav_e[:, c0 : c0 + ck])
                wd = pool.tile([WAV_ROWS, ck], BF16, tag="cd", name=f"wd{wi}")
                if sub_dve:
                    nc.vector.tensor_tensor(wd[:], cw[:], ew[:], OP.subtract)
                else:
                    nc.gpsimd.tensor_tensor(wd[:], cw[:], ew[:], OP.subtract)
                wjunk = pool.tile([WAV_ROWS, ck], BF16, tag="cjunk", bufs=1, name=f"wj{wi}")
                nc.vector.tensor_scalar(
                    wjunk[:], wd[:], 0.0, None, OP.abs_max,
                    accum_out=acc[:, (c := col("w")) : c + 1],
                )

            # ---- emission order: phases, coms, wav0, wav1 (split fine) ----
            w0, e1s, acc, halfpi, magic, negmagic = emit_consts()

            for k, (b, f0, P) in enumerate(
                [(b, f0, P) for b in range(BPC) for f0, P in ftiles]
            ):
                phase_pass(b, f0, P)

            com_list = [
                (b, c0) for b in range(BPC) for c0 in range(0, COM_COLS, COM_CHUNK)
            ]
            for i, (b, c0) in enumerate(com_list):
                com_pass(b, c0, sub_dve=(i in com_dve))

            # wav pass 0: whole 1600 cols; wav pass 1: split into tail_split
            wav_starts = [0, 1600]
            wav_pass(wav_starts[0], 1600, sub_dve=(0 in wav_dve))
            tail_ck = 1600 // tail_split
            for j in range(tail_split):
                wav_pass(wav_starts[1] + j * tail_ck, tail_ck, sub_dve=(1 in wav_dve))

            # ---- ship the whole accumulator; host reduces ----
            nc.sync.dma_start(out_d[:], acc[:])

    nc.compile()
    return nc


_CACHE = {}


def _get_nc():
    if "nc" not in _CACHE:
        _CACHE["nc"] = build_nc()
    return _CACHE["nc"]


def make_in_maps(inputs):
    """Slice the full inputs into per-core input maps."""
    clean_mag = np.asarray(inputs["clean_mag"], dtype=np.float32)
    enhan_mag = np.asarray(inputs["enhan_mag"], dtype=np.float32)
    clean_pha = np.asarray(inputs["clean_pha"], dtype=np.float32)
    clean_com = np.asarray(inputs["clean_com"], dtype=np.float32)
    enhan_com = np.asarray(inputs["enhan_com"], dtype=np.float32)
    clean_wav = np.asarray(inputs["clean_wav"], dtype=np.float32)
    enhan_wav = np.asarray(inputs["enhan_wav"], dtype=np.float32)

    in_maps = []
    for i in range(NCORES):
        sl = slice(BPC * i, BPC * (i + 1))
        in_maps.append(
            {
                "mag_c": np.ascontiguousarray(clean_mag[sl]),
                "mag_e": np.ascontiguousarray(enhan_mag[sl]),
                "pha_c": np.ascontiguousarray(clean_pha[sl]),
                "com_c": np.ascontiguousarray(clean_com[sl]).reshape(
                    BPC, COM_ROWS, COM_COLS
                ),
                "com_e": np.ascontiguousarray(enhan_com[sl]).reshape(
                    BPC, COM_ROWS, COM_COLS
                ),
                "wav_c": np.ascontiguousarray(clean_wav[sl]).reshape(
                    WAV_ROWS, WAV_COLS
                ),
                "wav_e": np.ascontiguousarray(enhan_wav[sl]).reshape(
                    WAV_ROWS, WAV_COLS
                ),
            }
        )
    return in_maps


def combine(partials, inputs):
    """Combine per-core partials ([NCORES, 128, NCOLS]) into the 6 losses."""
    p = np.asarray(partials, dtype=np.float64)
    p = p.reshape(-1, NCOLS).sum(axis=0)

    def tsum(term):
        return sum(p[c] for c in COLMAP[term])

    s_ip = tsum("ip")
    s_gd = tsum("gd")
    s_iaf = tsum("iaf")
    s_cos = tsum("cos")
    s_m2 = tsum("m2")
    s_c2 = tsum("c2")
    s_w = tsum("w")

    n = float(B * F * T)
    ip = TWO_PI_64 * s_ip / n
    # gd/iaf device cols hold sum(||y|-0.5|); dist(y) = 0.5 - ||y|-0.5|
    gd = TWO_PI_64 * (0.5 * n - s_gd) / n
    iaf = TWO_PI_64 * (0.5 * n - s_iaf) / n
    cspc = 1.0 - s_cos / n
    loss_mag = s_m2 / n
    loss_pha = ip + gd + iaf + cspc
    loss_com = 2.0 * s_c2 / (n * 2.0)
    loss_time = s_w / float(B * L)

    metric_g = np.asarray(inputs["metric_g"], dtype=np.float64).reshape(-1)
    one_labels = np.asarray(inputs["one_labels"], dtype=np.float64).reshape(-1)
    loss_metric = float(np.mean((metric_g - one_labels) ** 2))

    nloss = (
        loss_mag * 0.9
        + loss_pha * 0.3
        + loss_com * 0.1
        + loss_metric * 0.05
        + loss_time * 0.2
    )
    return tuple(
        np.float32(x)
        for x in (nloss, loss_mag, loss_pha, loss_com, loss_metric, loss_time)
    )


def _get_runner():
    """Build (once) a persistently-compiled 8-core sharded executor.

    Mirrors bass2jax.run_bass_via_pjrt but caches the jitted function so
    repeat calls skip retracing/recompiling. Returns
    (call(concat_inputs) -> partials[NCORES, 128, NCOLS], in_names,
    device_put_fn).
    """
    if "runner" in _CACHE:
        return _CACHE["runner"]
    import jax
    from concourse import bass2jax

    nc = _get_nc()
    bass2jax.install_neuronx_cc_hook()

    partition_name = nc.partition_id_tensor.name if nc.partition_id_tensor else None
    in_names, out_names, out_avals, zero_shapes = [], [], [], []
    for alloc in nc.m.functions[0].allocations:
        if not isinstance(alloc, mybir.MemoryLocationSet):
            continue
        name = alloc.memorylocations[0].name
        if alloc.kind == "ExternalInput":
            if name != partition_name:
                in_names.append(name)
        elif alloc.kind == "ExternalOutput":
            out_names.append(name)
            shape = tuple(alloc.tensor_shape)
            dtype = mybir.dt.np(alloc.dtype)
            out_avals.append(jax.core.ShapedArray(shape, dtype))
            zero_shapes.append((shape, dtype))
    n_params = len(in_names)
    all_in = list(in_names) + list(out_names)
    if partition_name is not None:
        all_in.append(partition_name)
    donate = tuple(range(n_params, n_params + len(out_names)))

    def _body(*args):
        operands = list(args)
        if partition_name is not None:
            operands.append(bass2jax.partition_id_tensor())
        outs = bass2jax._bass_exec_p.bind(
            *operands,
            out_avals=tuple(out_avals),
            in_names=tuple(all_in),
            out_names=tuple(out_names),
            lowering_input_output_aliases=(),
            sim_require_finite=True,
            sim_require_nnan=True,
            nc=nc,
        )
        return tuple(outs)

    devices = jax.devices()[:NCORES]
    mesh = bass2jax.Mesh(np.asarray(devices), ("core",))
    pspec = bass2jax.PartitionSpec("core")
    in_specs = (pspec,) * (n_params + len(out_names))
    out_specs = (pspec,) * len(out_names)
    sharded = jax.jit(
        bass2jax.shard_map(
            _body, mesh=mesh, in_specs=in_specs, out_specs=out_specs, check_rep=False
        ),
        donate_argnums=donate,
        keep_unused=True,
    )

    def make_zeros():
        return [
            np.zeros((NCORES * s[0], *s[1:]), d) for (s, d) in zero_shapes
        ]

    def call(concat_in):
        outs = sharded(*concat_in, *make_zeros())
        return np.asarray(outs[0]).reshape(NCORES, 128, NCOLS)

    def device_put(concat_in):
        sh = jax.sharding.NamedSharding(mesh, pspec)
        return [jax.device_put(a, sh) for a in concat_in]

    runner = (call, in_names, device_put, sharded, make_zeros)
    _CACHE["runner"] = runner
    return runner


def concat_inputs(in_maps, in_names):
    return [
        np.concatenate([m[name] for m in in_maps], axis=0) for name in in_names
    ]


def run(inputs):
    in_maps = make_in_maps(inputs)
    try:
        call, in_names, _, _, _ = _get_runner()
        partials = call(concat_inputs(in_maps, in_names))
    except Exception:
        nc = _get_nc()
        res = run_bass_kernel_spmd(nc, in_maps, core_ids=list(range(NCORES)))
        partials = np.asarray([r["partials"] for r in res.results])
    return combine(partials, inputs)


def kernel(**inputs):
    return run(inputs)


# revision 13
# speedup vs baseline: 1.0303x; 1.0303x over previous
"""Trainium2 Bass kernel for the speech-enhancement loss function.

Math (matching the jax reference):
  loss_mag    = mean((clean_mag - enhan_mag)^2)
  d           = clean_pha - enhan_mag          (reference quirk: enhan_mag is phase_g)
  ip_loss     = mean(aw(d)),   aw(x) = |x - round(x/2pi)*2pi|
  gd_loss     = mean(aw(gd)),  gd[:,0,:] = -d[:,0,:]; gd[:,j,:] = d[:,j-1,:]-d[:,j,:]
  iaf_loss    = mean(aw(iaf)), same shifted difference along the T axis
  cspc_loss   = mean(1 - cos(aw(d))) = mean(1 - cos(d))
  loss_com    = mean((clean_com - enhan_com)^2) * 2
  loss_time   = mean(|clean_wav - enhan_wav|)
  loss_metric = mean((metric_g - 1)^2)            (tiny -> host)

Sharding: data-parallel over the batch dim, 2 batches per core on 8 cores.
Each core computes partial SUMS of each term into an SBUF accumulator
acc[128, NCOLS]; the full accumulator ships out and the host does the final
partition/column sums in float64.

The kernel is DMA-bound: 26.33 MB/core of fp32 input at the cost model's
360 B/ns DMA floor = 73.1 us. The design goal is every compute engine well
under that floor so the schedule is a single gapless DMA stream plus a
short tail. Engine budget (per core):
  DVE ~39us, ACT ~33us, Pool(GPSIMD) ~49us, PE ~8us.

Key tricks:
  * round(q) via the fp32 magic constant: v = q + 1.5*2^23; r = v - MAGIC.
    Both run on ACT as Identity(scale*x + bias) (exact fp32 in the interp),
    freeing DVE.
  * everything downstream of f = q - round(q) is bf16: DVE tensor_scalar /
    scalar_tensor_tensor ops with all-bf16 SBUF operands run at 4x
    (tensor_tensor at 2x), and the PE banded matmul for gd runs at 1
    cycle/row instead of fp32's 4.
  * abs on DVE: abs_max(x, 0.0). The anti-wrap distance sum for gd/iaf,
    sum ||y|-0.5|, fuses into tensor_scalar((y abs_max 0) add -0.5) +
    tensor_scalar(abs_max, accum_out) pairs; ip's sum |f| is a single
    fused abs+accum whose output |f| also feeds ACT's Sin for cspc
    (cos(d) = sin(pi/2 - 2pi|f|), keeping the Sin arg in [-pi/2, pi/2]).
  * dist(y) = 0.5 - ||y|-0.5| is 1-periodic in y, so bf16 rounding of f
    (and any round-boundary flips) shifts y by integers and cancels.
  * per-term partial sums land in per-instruction accumulator columns
    (fp32, accum_out); the host combines, so no on-device final reduce
    sits on the critical tail.
Ordering: all phase passes first (their deep DVE->ACT->DVE chains digest
while com/wav DMAs stream), then com chunks, then wav with a small final
DVE-subtracted chunk so the post-DMA tail is ~1us of compute + the
unavoidable ~2.8us output-DMA/drain chain.
"""

import numpy as np

import concourse.bacc as bacc
import concourse.mybir as mybir
import concourse.tile as tile
from concourse.bass_utils import run_bass_kernel_spmd

F32 = mybir.dt.float32
BF16 = mybir.dt.bfloat16
OP = mybir.AluOpType
AF = mybir.ActivationFunctionType

B, F, T, L = 16, 201, 2048, 204800
NCORES = 8
BPC = B // NCORES  # batches per core

TWO_PI_64 = 2.0 * np.pi
S = float(np.float32(1.0) / np.float32(TWO_PI_64))  # 1/(2pi) in fp32
MAGIC = float(np.float32(1.5 * 2**23))  # 12582912.0, round-to-int trick
HALF_PI = float(np.float32(np.pi / 2))
NEG_TWO_PI = float(np.float32(-TWO_PI_64))

# com per core: BPC*F*T*2 = 1646592 = 2 batches x (128 x 6432)
COM_ROWS, COM_COLS = 128, 6432
COM_CHUNK = 1608  # 4 chunks per batch
# wav per core: BPC*L = 409600 = 128 x 3200
WAV_ROWS, WAV_COLS = 128, 3200

NCOLS = 96  # accumulator columns

# term -> list of acc columns, populated by build_nc (deterministic)
COLMAP = {}


def _w0_matrix():
    # lhsT[k, j] = delta_{j,k+1} - delta_{j,k}  ->  (W0 @ f)[j] = f[j-1] - f[j]
    w = np.zeros((128, 128), dtype=np.float32)
    for k in range(128):
        w[k, k] = -1.0
        if k + 1 < 128:
            w[k, k + 1] = 1.0
    return w


def _e127_matrix():
    # lhsT[k, j] = delta_{k,127} delta_{j,0}: selects f_prev row 127 into
    # output partition 0 (and zero elsewhere) -- boundary patch without any
    # SBUF->SBUF DMA.
    e = np.zeros((128, 128), dtype=np.float32)
    e[127, 0] = 1.0
    return e


def build_nc(
    nch=1,            # T-chunks per phase pass (pipeline the serial chain)
    in_bufs=3,        # cm/em double-buffer depth
    cp_bufs=2,        # cp buffer depth
    com_bufs=3,       # com/wav input buffer depth
    qg_chunks=2,      # gd PSUM halves
    mag_dve=(),       # phase pass indices whose mag sub runs on DVE (else Pool)
    m2_act=(),        # phase pass indices whose m^2 sum runs on ACT Square
    tail_sizes=(804, 402, 402),  # final com chunk split (sums to 1608)
):
    nc = bacc.Bacc(None, target_bir_lowering=False)

    mag_c = nc.dram_tensor("mag_c", [BPC, F, T], F32, kind="ExternalInput")
    mag_e = nc.dram_tensor("mag_e", [BPC, F, T], F32, kind="ExternalInput")
    pha_c = nc.dram_tensor("pha_c", [BPC, F, T], F32, kind="ExternalInput")
    com_c = nc.dram_tensor("com_c", [BPC, COM_ROWS, COM_COLS], F32, kind="ExternalInput")
    com_e = nc.dram_tensor("com_e", [BPC, COM_ROWS, COM_COLS], F32, kind="ExternalInput")
    wav_c = nc.dram_tensor("wav_c", [WAV_ROWS, WAV_COLS], F32, kind="ExternalInput")
    wav_e = nc.dram_tensor("wav_e", [WAV_ROWS, WAV_COLS], F32, kind="ExternalInput")
    out_d = nc.dram_tensor("partials", [128, NCOLS], F32, kind="ExternalOutput")

    import ml_dtypes

    w0_d = nc.inline_tensor(_w0_matrix().astype(ml_dtypes.bfloat16), name="w0shift")
    e127_d = nc.inline_tensor(_e127_matrix().astype(ml_dtypes.bfloat16), name="e127row")

    COLMAP.clear()
    _next_col = [0]

    def col(term):
        c = _next_col[0]
        _next_col[0] += 1
        assert c < NCOLS
        COLMAP.setdefault(term, []).append(c)
        return c

    with tile.TileContext(nc) as tc:
        with (
            tc.tile_pool(name="main", bufs=2) as pool,
            tc.tile_pool(name="psum", bufs=1, space="PSUM") as psum,
        ):
            ftiles = [(0, 128), (128, 73)]  # (f0, P)
            f_prev_by_b = {}
            counters = {"pi": 0, "ci": 0, "wi": 0}
            phase_in = {}

            def emit_inputs(pi, b, f0, P):
                cm = pool.tile([P, T], F32, tag="in_a", bufs=in_bufs, name=f"cm{pi}")
                nc.sync.dma_start(cm[:], mag_c[b, f0 : f0 + P, :])
                em = pool.tile([P, T], F32, tag="in_b", bufs=in_bufs, name=f"em{pi}")
                nc.sync.dma_start(em[:], mag_e[b, f0 : f0 + P, :])
                cp = pool.tile([P, T], F32, tag="in_c", bufs=cp_bufs, name=f"cp{pi}")
                nc.sync.dma_start(cp[:], pha_c[b, f0 : f0 + P, :])
                phase_in[pi] = (cm, em, cp)

            def emit_consts():
                w0 = pool.tile([128, 128], BF16, tag="w0", bufs=1)
                nc.sync.dma_start(w0[:], w0_d[:])
                e127 = pool.tile([128, 128], BF16, tag="e127", bufs=1)
                nc.sync.dma_start(e127[:], e127_d[:])
                acc = pool.tile([128, NCOLS], F32, tag="acc", bufs=1)
                nc.vector.memset(acc[:], 0.0)
                halfpi = pool.tile([128, 1], F32, tag="halfpi", bufs=1)
                nc.vector.memset(halfpi[:], HALF_PI)
                magic = pool.tile([128, 1], F32, tag="magic", bufs=1)
                nc.vector.memset(magic[:], MAGIC)
                negmagic = pool.tile([128, 1], F32, tag="negmagic", bufs=1)
                nc.vector.memset(negmagic[:], -MAGIC)
                return w0, e127, acc, halfpi, magic, negmagic

            def phase_pass(pi, b, f0, P):
                f_prev = f_prev_by_b.get(b)
                cm, em, cp = phase_in[pi]
                CT = T // nch
                d = pool.tile([P, T], F32, tag="d", name=f"d{pi}")
                v = pool.tile([P, T], F32, tag="v", name=f"v{pi}")
                f = pool.tile([P, T], BF16, tag="f", name=f"f{pi}")
                af = pool.tile([P, T], BF16, tag="af", bufs=1, name=f"af{pi}")
                js = pool.tile([P, T], BF16, tag="js", bufs=1, name=f"js{pi}")
                fd = pool.tile([P, T], BF16, tag="fd", name=f"fd{pi}")
                at = pool.tile([P, T], BF16, tag="at", name=f"at{pi}")
                zt = pool.tile([P, T], BF16, tag="zt", bufs=1, name=f"zt{pi}")
                chunks = [slice(c0, c0 + CT) for c0 in range(0, T, CT)]

                # round chain: d (DVE) -> v, r (ACT, r in-place) -> f (DVE)
                for ts_ in chunks:
                    nc.vector.tensor_tensor(d[:, ts_], cp[:, ts_], em[:, ts_], OP.subtract)
                for ts_ in chunks:
                    nc.scalar.activation(
                        v[:, ts_], d[:, ts_], AF.Identity, bias=magic[0:P, :], scale=S
                    )
                for ts_ in chunks:
                    nc.scalar.activation(
                        v[:, ts_], v[:, ts_], AF.Identity, bias=negmagic[0:P, :]
                    )
                for ts_ in chunks:
                    nc.vector.scalar_tensor_tensor(
                        f[:, ts_], d[:, ts_], S, v[:, ts_], OP.mult, OP.subtract
                    )
                # ip: af = |f| on ACT with accum (feeds Sin); cspc: sin accum
                for ts_ in chunks:
                    nc.scalar.activation(
                        af[:, ts_], f[:, ts_], AF.Abs,
                        accum_out=acc[0:P, (c := col("ip")) : c + 1],
                    )
                for ts_ in chunks:
                    nc.scalar.activation(
                        js[:, ts_], af[:, ts_], AF.Sin, bias=halfpi[0:P, :],
                        scale=NEG_TWO_PI,
                        accum_out=acc[0:P, (c := col("cos")) : c + 1],
                    )
                # mag: m = cm - em (bf16); sum m^2: DVE tt-mult + cache-reduce
                # (in-place), or ACT Square+accum for m2_act passes
                m = pool.tile([P, T], BF16, tag="m", name=f"m{pi}")
                if pi in mag_dve:
                    nc.vector.tensor_tensor(m[:], cm[:], em[:], OP.subtract)
                else:
                    nc.gpsimd.tensor_tensor(m[:], cm[:], em[:], OP.subtract)
                if pi in m2_act:
                    mj = pool.tile([P, T], BF16, tag="mj", bufs=1, name=f"mj{pi}")
                    nc.scalar.activation(
                        mj[:], m[:], AF.Square,
                        accum_out=acc[0:P, (c := col("m2")) : c + 1],
                    )
                else:
                    z = pool.tile([P, T], BF16, tag="z", bufs=1, name=f"z{pi}")
                    nc.vector.tensor_tensor(z[:], m[:], m[:], OP.mult)
                    nc.vector.tensor_scalar(
                        z[:], z[:], 0.0, 0.0, OP.add, OP.add,
                        accum_out=acc[0:P, (c := col("m2")) : c + 1],
                    )
                # iaf: fd = shifted diff (DVE 2x); at = |fd| on ACT with accum
                # (sum |y|); zt = at - 0.5 (4x); relu+accum (4x, in-place).
                # dist(y) = |y| - 2*relu(|y| - 0.5)
                for ts_ in chunks:
                    tc0 = ts_.start
                    lo = tc0 if tc0 else 1
                    if tc0 == 0:
                        nc.vector.tensor_copy(fd[:, 0:1], f[:, 0:1])
                    nc.vector.tensor_tensor(
                        fd[:, lo : ts_.stop], f[:, lo - 1 : ts_.stop - 1],
                        f[:, lo : ts_.stop], OP.subtract
                    )
                for ts_ in chunks:
                    nc.scalar.activation(
                        at[:, ts_], fd[:, ts_], AF.Abs,
                        accum_out=acc[0:P, (c := col("iafa")) : c + 1],
                    )
                for ts_ in chunks:
                    nc.vector.tensor_scalar(zt[:, ts_], at[:, ts_], -0.5, None, OP.add)
                    nc.vector.tensor_scalar(
                        zt[:, ts_], zt[:, ts_], 0.0, 0.0, OP.max, OP.add,
                        accum_out=acc[0:P, (c := col("iafr")) : c + 1],
                    )
                # gd via PE banded mm (bf16, e127 patch); same relu-dist sums
                HT = T // qg_chunks
                for h in range(qg_chunks):
                    qg = psum.tile([P, HT], F32, tag="qg", bufs=2, name=f"qg{pi}_{h}")
                    for n0 in range(0, HT, 512):
                        nn = h * HT + n0
                        if f_prev is None:
                            nc.tensor.matmul(
                                qg[:, n0 : n0 + 512], w0[0:P, 0:P],
                                f[:, nn : nn + 512],
                            )
                        else:
                            nc.tensor.matmul(
                                qg[:, n0 : n0 + 512], w0[0:P, 0:P],
                                f[:, nn : nn + 512], start=True, stop=False,
                            )
                            nc.tensor.matmul(
                                qg[:, n0 : n0 + 512], e127[0:128, 0:P],
                                f_prev[:, nn : nn + 512], start=False, stop=True,
                            )
                    ag = pool.tile([P, HT], BF16, tag="ag", name=f"ag{pi}_{h}")
                    nc.scalar.activation(
                        ag[:], qg[:], AF.Abs,
                        accum_out=acc[0:P, (c := col("gda")) : c + 1],
                    )
                    gt = pool.tile([P, HT], BF16, tag="gt", name=f"gt{pi}_{h}")
                    nc.vector.tensor_scalar(gt[:], ag[:], -0.5, None, OP.add)
                    nc.vector.tensor_scalar(
                        gt[:], gt[:], 0.0, 0.0, OP.max, OP.add,
                        accum_out=acc[0:P, (c := col("gdr")) : c + 1],
                    )
                f_prev_by_b[b] = f

            def com_pass(b, c0, ck, sub_dve=False):
                ci = counters["ci"]
                counters["ci"] += 1
                cc = pool.tile([COM_ROWS, ck], F32, tag="com_a", bufs=com_bufs, name=f"cc{ci}")
                nc.sync.dma_start(cc[:], com_c[b, :, c0 : c0 + ck])
                ec = pool.tile([COM_ROWS, ck], F32, tag="com_b", bufs=com_bufs, name=f"ec{ci}")
                nc.sync.dma_start(ec[:], com_e[b, :, c0 : c0 + ck])
                cd = pool.tile([COM_ROWS, ck], BF16, tag="cd", name=f"cd{ci}")
                if sub_dve:
                    nc.vector.tensor_tensor(cd[:], cc[:], ec[:], OP.subtract)
                else:
                    nc.gpsimd.tensor_tensor(cd[:], cc[:], ec[:], OP.subtract)
                cz = pool.tile([COM_ROWS, ck], BF16, tag="cz", bufs=1, name=f"cz{ci}")
                nc.vector.tensor_tensor(cz[:], cd[:], cd[:], OP.mult)
                nc.vector.tensor_scalar(
                    cz[:], cz[:], 0.0, 0.0, OP.add, OP.add,
                    accum_out=acc[:, (c := col("c2")) : c + 1],
                )

            def wav_pass(c0, ck, sub_dve=False):
                wi = counters["wi"]
                counters["wi"] += 1
                cw = pool.tile([WAV_ROWS, ck], F32, tag="com_a", bufs=com_bufs, name=f"cw{wi}")
                nc.sync.dma_start(cw[:], wav_c[:, c0 : c0 + ck])
                ew = pool.tile([WAV_ROWS, ck], F32, tag="com_b", bufs=com_bufs, name=f"ew{wi}")
                nc.sync.dma_start(ew[:], wav_e[:, c0 : c0 + ck])
                wd = pool.tile([WAV_ROWS, ck], BF16, tag="cd", name=f"wd{wi}")
                if sub_dve:
                    nc.vector.tensor_tensor(wd[:], cw[:], ew[:], OP.subtract)
                else:
                    nc.gpsimd.tensor_tensor(wd[:], cw[:], ew[:], OP.subtract)
                wj = pool.tile([WAV_ROWS, ck], BF16, tag="wj", bufs=1, name=f"wj{wi}")
                nc.scalar.activation(
                    wj[:], wd[:], AF.Abs,
                    accum_out=acc[:, (c := col("w")) : c + 1],
                )

            # ---- emission: input DMAs first (SP order = pure input stream),
            # then phase compute, then coms/wavs with a shrinking DVE tail ----
            plist = [(b, f0, P) for b in range(BPC) for f0, P in ftiles]
            emit_inputs(0, *plist[0])
            w0, e127, acc, halfpi, magic, negmagic = emit_consts()
            for pi in range(1, 4):
                emit_inputs(pi, *plist[pi])
            for pi in range(4):
                phase_pass(pi, *plist[pi])

            # com chunks: batch 0 whole, batch 1 ends with the split tail
            for c0 in range(0, COM_COLS, COM_CHUNK):
                com_pass(0, c0, COM_CHUNK)
            n_main = COM_COLS - sum(tail_sizes)
            for c0 in range(0, n_main, COM_CHUNK):
                com_pass(1, c0, COM_CHUNK)
            # wavs (Pool subs, ACT abs+accum), then the all-DVE com tail
            wav_pass(0, 1600)
            wav_pass(1600, 1600)
            c0 = n_main
            for ck in tail_sizes:
                com_pass(1, c0, ck, sub_dve=True)
                c0 += ck

            # ---- ship the whole accumulator; host reduces ----
            nc.sync.dma_start(out_d[:], acc[:])

    nc.compile()
    return nc


_CACHE = {}


def _get_nc():
    if "nc" not in _CACHE:
        _CACHE["nc"] = build_nc()
    return _CACHE["nc"]


def make_in_maps(inputs):
    """Slice the full inputs into per-core input maps."""
    clean_mag = np.asarray(inputs["clean_mag"], dtype=np.float32)
    enhan_mag = np.asarray(inputs["enhan_mag"], dtype=np.float32)
    clean_pha = np.asarray(inputs["clean_pha"], dtype=np.float32)
    clean_com = np.asarray(inputs["clean_com"], dtype=np.float32)
    enhan_com = np.asarray(inputs["enhan_com"], dtype=np.float32)
    clean_wav = np.asarray(inputs["clean_wav"], dtype=np.float32)
    enhan_wav = np.asarray(inputs["enhan_wav"], dtype=np.float32)

    in_maps = []
    for i in range(NCORES):
        sl = slice(BPC * i, BPC * (i + 1))
        in_maps.append(
            {
                "mag_c": np.ascontiguousarray(clean_mag[sl]),
                "mag_e": np.ascontiguousarray(enhan_mag[sl]),
                "pha_c": np.ascontiguousarray(clean_pha[sl]),
                "com_c": np.ascontiguousarray(clean_com[sl]).reshape(
                    BPC, COM_ROWS, COM_COLS
                ),
                "com_e": np.ascontiguousarray(enhan_com[sl]).reshape(
                    BPC, COM_ROWS, COM_COLS
                ),
                "wav_c": np.ascontiguousarray(clean_wav[sl]).reshape(
                    WAV_ROWS, WAV_COLS
                ),
                "wav_e": np.ascontiguousarray(enhan_wav[sl]).reshape(
                    WAV_ROWS, WAV_COLS
                ),
            }
        )
    return in_maps


def combine(partials, inputs):
    """Combine per-core partials ([NCORES, 128, NCOLS]) into the 6 losses."""
    p = np.asarray(partials, dtype=np.float64)
    p = p.reshape(-1, NCOLS).sum(axis=0)

    def tsum(term):
        return sum(p[c] for c in COLMAP[term])

    s_ip = tsum("ip")
    s_cos = tsum("cos")
    s_m2 = tsum("m2")
    s_c2 = tsum("c2")
    s_w = tsum("w")

    n = float(B * F * T)
    ip = TWO_PI_64 * s_ip / n
    # gd/iaf: device cols hold sum|y| and sum relu(|y|-0.5);
    # dist(y) = |y| - 2*relu(|y|-0.5)
    gd = TWO_PI_64 * (tsum("gda") - 2.0 * tsum("gdr")) / n
    iaf = TWO_PI_64 * (tsum("iafa") - 2.0 * tsum("iafr")) / n
    cspc = 1.0 - s_cos / n
    loss_mag = s_m2 / n
    loss_pha = ip + gd + iaf + cspc
    loss_com = 2.0 * s_c2 / (n * 2.0)
    loss_time = s_w / float(B * L)

    metric_g = np.asarray(inputs["metric_g"], dtype=np.float64).reshape(-1)
    one_labels = np.asarray(inputs["one_labels"], dtype=np.float64).reshape(-1)
    loss_metric = float(np.mean((metric_g - one_labels) ** 2))

    nloss = (
        loss_mag * 0.9
        + loss_pha * 0.3
        + loss_com * 0.1
        + loss_metric * 0.05
        + loss_time * 0.2
    )
    return tuple(
        np.float32(x)
        for x in (nloss, loss_mag, loss_pha, loss_com, loss_metric, loss_time)
    )


def _get_runner():
    """Build (once) a persistently-compiled 8-core sharded executor.

    Mirrors bass2jax.run_bass_via_pjrt but caches the jitted function so
    repeat calls skip retracing/recompiling. Returns
    (call(concat_inputs) -> partials[NCORES, 128, NCOLS], in_names,
    device_put_fn).
    """
    if "runner" in _CACHE:
        return _CACHE["runner"]
    import jax
    from concourse import bass2jax

    nc = _get_nc()
    bass2jax.install_neuronx_cc_hook()

    partition_name = nc.partition_id_tensor.name if nc.partition_id_tensor else None
    in_names, out_names, out_avals, zero_shapes = [], [], [], []
    for alloc in nc.m.functions[0].allocations:
        if not isinstance(alloc, mybir.MemoryLocationSet):
            continue
        name = alloc.memorylocations[0].name
        if alloc.kind == "ExternalInput":
            if name != partition_name:
                in_names.append(name)
        elif alloc.kind == "ExternalOutput":
            out_names.append(name)
            shape = tuple(alloc.tensor_shape)
            dtype = mybir.dt.np(alloc.dtype)
            out_avals.append(jax.core.ShapedArray(shape, dtype))
            zero_shapes.append((shape, dtype))
    n_params = len(in_names)
    all_in = list(in_names) + list(out_names)
    if partition_name is not None:
        all_in.append(partition_name)
    donate = tuple(range(n_params, n_params + len(out_names)))

    def _body(*args):
        operands = list(args)
        if partition_name is not None:
            operands.append(bass2jax.partition_id_tensor())
        outs = bass2jax._bass_exec_p.bind(
            *operands,
            out_avals=tuple(out_avals),
            in_names=tuple(all_in),
            out_names=tuple(out_names),
            lowering_input_output_aliases=(),
            sim_require_finite=True,
            sim_require_nnan=True,
            nc=nc,
        )
        return tuple(outs)

    devices = jax.devices()[:NCORES]
    mesh = bass2jax.Mesh(np.asarray(devices), ("core",))
    pspec = bass2jax.PartitionSpec("core")
    in_specs = (pspec,) * (n_params + len(out_names))
    out_specs = (pspec,) * len(out_names)
    sharded = jax.jit(
        bass2jax.shard_map(
            _body, mesh=mesh, in_specs=in_specs, out_specs=out_specs, check_rep=False
        ),
        donate_argnums=donate,
        keep_unused=True,
    )

    def make_zeros():
        return [
            np.zeros((NCORES * s[0], *s[1:]), d) for (s, d) in zero_shapes
        ]

    def call(concat_in):
        outs = sharded(*concat_in, *make_zeros())
        return np.asarray(outs[0]).reshape(NCORES, 128, NCOLS)

    def device_put(concat_in):
        sh = jax.sharding.NamedSharding(mesh, pspec)
        return [jax.device_put(a, sh) for a in concat_in]

    runner = (call, in_names, device_put, sharded, make_zeros)
    _CACHE["runner"] = runner
    return runner


def concat_inputs(in_maps, in_names):
    return [
        np.concatenate([m[name] for m in in_maps], axis=0) for name in in_names
    ]


def run(inputs):
    in_maps = make_in_maps(inputs)
    try:
        call, in_names, _, _, _ = _get_runner()
        partials = call(concat_inputs(in_maps, in_names))
    except Exception:
        nc = _get_nc()
        res = run_bass_kernel_spmd(nc, in_maps, core_ids=list(range(NCORES)))
        partials = np.asarray([r["partials"] for r in res.results])
    return combine(partials, inputs)


def kernel(**inputs):
    return run(inputs)
